# revision 28
# baseline (speedup 1.0000x reference)
"""Trainium2 Bass kernel for nn_BlockRF (BatchNorm -> LocallyConnected2D 3x3 valid -> ReLU).

Shapes (hardcoded per the problem spec):
  x:      [B=32, H=64, W=64, C=32]  f32
  gamma/beta/moving_mean/moving_var: [C=32] f32
  weight: [OH*OW=3844, KH*KW*C=288, F=32] f32
  out:    [B=32, OH=62, OW=62, F=32] f32

Strategy: shard over output rows (OH). OH=62 padded to 64 -> 8 rows/core on 8
cores. Each core streams only its slice of the (dominant) weight tensor.

The binding resource is SDMA byte throughput (~200 GB/s/core on SBUF-side
bytes, measured), so the weight stream is stored fp8 E3M4 (float8e3) with one
global symmetric scale sw (host quantized) and consumed directly by a
mixed-precision matmul (fp16 stationary x, fp8e3 moving weights — HW
supported, bit-exact). sw is folded into the BN affine params
(xn_scaled = sw*(A*x+Bb)), so the matmul directly produces y and no dequant
op exists anywhere.

Per core, per output row oh (pipelined via tile pools):
  - x rows r0..r0+2 live in an SBUF tile X[(i,c)=96, (w,b)=2048] fp16, BN
    (pre-scaled by sw) applied by VectorE; x loads ride the gpsimd SWDGE
    ring alone, y stores the tails of the two HWDGE rings (so stores never
    head-of-line block a weight prefetch; no gpsimd tensor ops - they
    collide with DVE SBUF access and slow both engines).
  - weights stream per-oh: W[(i,c)=96, (w,g,f)=5952] fp8e3, slot (w,g) holds
    the 3x32-channel chunk j=2-g of position ow=w-2+g; each load is split
    across the sync and scalar HWDGE rings.
  - For each position ow: 3 accumulating matmuls (K=96, M=B=32, N<=3*F);
    lhsT = X w-slice (stationary fp16), rhs = weight chunk (moving fp8e3),
    fp32 PSUM accumulation.
  - PSUM: ONE [128, 512] single-bank tile per oh; the 4 position groups of
    16 live at partition offsets 0/32/64/96 (PE col tiling, tile_position
    auto-derived from out.base_partition). One [128,512] memset replaces two
    expensive 32-partition memsets, and matmuls of different groups can
    overlap in the PE array.
  - ReLU evacuation (fp16 out): 4 slices [32,512], alternating Vector/Scalar.
  - a short warmup burst of K=96 zero-matmuls at kernel start keeps the
    PE HAM clock gate (1.2 -> 2.4 GHz) favorably phased while the first
    DMAs stream.

Host side only pads/transposes/casts/quantizes (layout prep + sharding) - all
model arithmetic (BN, conv, ReLU) runs on device.
"""

import numpy as np

B, H, W, C, F = 32, 64, 64, 32, 32
KH = KW = 3
OH = OW = 62
OHP = 64  # padded OH
RPC = OHP // 8  # output rows per core = 8
EPS = 1e-3
NPART = KH * C  # 96 partitions = (i, c)
XFREE = W * B  # 2048
FP8_MAX = 15.5  # max finite |value| of E3M4
WARMUP_MM = 4  # K=96 N=512 zero-matmuls to lift the HAM clock gate
# packed weight slots: for stationary w-slice, valid g values are those with
# ow = w-2+g in [0, OW); slots stored g-ascending, w-major
_GMIN = [max(0, 2 - w) for w in range(W)]
_GMAX = [min(2, OW - 1 - (w - 2)) for w in range(W)]
_SLOT_BASE = [0] * W
for _w in range(1, W):
    _SLOT_BASE[_w] = _SLOT_BASE[_w - 1] + (_GMAX[_w - 1] - _GMIN[_w - 1] + 1)
NSLOT = _SLOT_BASE[-1] + (_GMAX[-1] - _GMIN[-1] + 1)  # 186
WSLOT = NSLOT * F  # 5952
YFREE = OW * F  # 1984
PSUM_POS = 16  # positions per PSUM partition-group (16*32*4B = 2KB = 1 bank)

_CACHE = {}


def _build_program():
    import concourse.mybir as mybir
    import concourse.tile as tile
    from concourse import bacc
    from contextlib import ExitStack

    f16 = mybir.dt.float16
    f32 = mybir.dt.float32
    f8 = mybir.dt.float8e3

    nc = bacc.Bacc("TRN2", target_bir_lowering=False, debug=False, num_devices=8)

    xin = nc.dram_tensor("xin", [RPC + 2, C, XFREE], f16, kind="ExternalInput").ap()
    win = nc.dram_tensor("win", [RPC, NPART, WSLOT], f8, kind="ExternalInput").ap()
    pin = nc.dram_tensor("pin", [NPART, 5], f32, kind="ExternalInput").ap()
    yout = nc.dram_tensor("yout", [RPC, B, YFREE], f16, kind="ExternalOutput").ap()

    ngrp = (OW + PSUM_POS - 1) // PSUM_POS  # 4 groups of <=16 positions

    with ExitStack() as ctx:
        tc = ctx.enter_context(tile.TileContext(nc))
        singles = ctx.enter_context(tc.tile_pool(name="singles", bufs=1))
        xnpool = ctx.enter_context(tc.tile_pool(name="xnpool", bufs=3))
        wpool = ctx.enter_context(tc.tile_pool(name="wpool", bufs=12))
        opool = ctx.enter_context(tc.tile_pool(name="opool", bufs=8))
        pspool = ctx.enter_context(
            tc.tile_pool(name="pspool", bufs=6, space="PSUM")
        )

        # ---- BN affine params scaled by the global weight dequant scale sw
        # (pin col 4): A = sw*gamma/sqrt(var+eps), Bb = sw*(beta - mean*A0)
        par = singles.tile([NPART, 5], f32)
        nc.sync.dma_start(out=par, in_=pin)

        # x dedup via two rotating 3-row stores (even rows use xse, odd
        # xso); input row l sits at partition block (l mod 3). Each store
        # replaces its two oldest rows right after its BN runs - two whole
        # row-periods before the refreshed rows are needed. The gpsimd ring
        # carries ONLY these loads, in monotone dependency order (BNs
        # complete in row order), so its FIFO never inverts. x SBUF
        # traffic: 18 row-loads instead of 24.
        xse = singles.tile([NPART, XFREE], f16, name="xse")
        xso = singles.tile([NPART, XFREE], f16, name="xso")
        xstores = [xse, xso]

        def load_rows(store, l0, k):
            # rows l0..l0+k-1 -> partition blocks (l mod 3); one DMA per
            # contiguous block run
            l = l0
            while l < l0 + k:
                b = l % 3
                run = 1
                while l + run < l0 + k and b + run < 3:
                    run += 1
                nc.gpsimd.dma_start(
                    out=store[32 * b:32 * (b + run), :],
                    in_=xin[l:l + run],
                )
                l += run

        load_rows(xse, 0, 3)
        load_rows(xso, 1, 3)

        # PE warmup: K=96 zero-matmuls keep the PE array visibly busy while
        # the first DMAs stream, so the HAM clock gate opens (1.2 -> 2.4
        # GHz) before the real matmuls start (K=1 matmuls do NOT register
        # enough PE activity to trip the monitor)
        zt = singles.tile([NPART, 512], f16)
        nc.vector.memset(zt, 0.0)
        wups = pspool.tile([128, 512], mybir.dt.float32, name="wup", tag="ps")
        for _ in range(WARMUP_MM):
            nc.tensor.matmul(
                wups[:B], zt[:, :B], zt, start=True, stop=True,
                skip_group_check=True,
            )

        tmp = singles.tile([NPART, 1], f32)
        A = singles.tile([NPART, 1], f32)
        Bb = singles.tile([NPART, 1], f32)
        nc.vector.tensor_scalar_add(tmp, par[:, 3:4], EPS)  # var + eps
        nc.scalar.sqrt(tmp, tmp)
        nc.vector.reciprocal(A, tmp)  # 1/sqrt(var+eps)
        nc.vector.tensor_mul(A, A, par[:, 0:1])  # * gamma
        nc.vector.tensor_mul(tmp, A, par[:, 2:3])  # mean * A
        nc.vector.tensor_sub(Bb, par[:, 1:2], tmp)  # beta - mean*A
        nc.vector.tensor_mul(A, A, par[:, 4:5])  # * sw
        nc.vector.tensor_mul(Bb, Bb, par[:, 4:5])  # * sw

        HW = WSLOT // 2  # = 93*F: exactly the slots of w<32
        rowbufs = []
        for oh in range(RPC):
            # two weight tiles per row (w<32 / w>=32), loaded SEQUENTIALLY
            # on one HWDGE ring (even rows on sync, odd on scalar): the
            # front tile lands half a period earlier, so each row's matmul
            # burst splits into two sub-bursts ~2.2us apart - the PE HAM
            # activity window never sees a long idle stretch (which would
            # re-throttle the clock to 1.2 GHz), and the first matmuls of
            # each row start earlier
            weng = nc.sync if oh % 2 == 0 else nc.scalar
            wta = wpool.tile([NPART, HW], f8, name="wta", tag="wt")
            wtb = wpool.tile([NPART, WSLOT - HW], f8, name="wtb", tag="wt")
            weng.dma_start(out=wta, in_=win[oh][:, :HW])
            weng.dma_start(out=wtb, in_=win[oh][:, HW:])
            xt = xstores[oh % 2]
            xn = xnpool.tile([NPART, XFREE], f16)
            nc.vector.tensor_scalar(
                xn, xt, A, Bb,
                op0=mybir.AluOpType.mult, op1=mybir.AluOpType.add,
            )
            if oh + 2 < RPC:
                # refresh this store for its next use (rows oh+3, oh+4)
                load_rows(xstores[oh % 2], oh + 3, 2)

            rowbuf = opool.tile([B, YFREE], f16)
            # ONE single-bank PSUM tile; position group g lives at
            # partition offset 32g (PE col tiling). PSUM 'start=True'
            # pend-zeroes a whole 2KB bank, so interleaved accumulation
            # slices cannot use it: memset the tile instead and accumulate
            # every matmul (start=False onto zeroed values).
            pst = pspool.tile([128, PSUM_POS * F], mybir.dt.float32,
                              name="ps", tag="ps")
            nc.vector.memset(pst, 0.0)

            def emit(w, ow_lo, ow_hi):
                # one matmul covering positions ow_lo..ow_hi (inclusive) at
                # stationary w-slice; slots (w, g=2-(w-ow)) are
                # free-contiguous for ascending ow
                grp = ow_lo // PSUM_POS
                s = ow_lo - grp * PSUM_POS
                n = ow_hi - ow_lo + 1
                g_lo = 2 - (w - ow_lo)
                slot = _SLOT_BASE[w] + (g_lo - _GMIN[w])
                if slot * F < HW:
                    wsrc = wta[:, slot * F:(slot + n) * F]
                else:
                    wsrc = wtb[:, slot * F - HW:(slot + n) * F - HW]
                nc.tensor.matmul(
                    pst[32 * grp:32 * grp + B, s * F:(s + n) * F],
                    xn[:, w * B:(w + 1) * B],
                    wsrc,
                    start=False,
                    stop=True,
                    skip_group_check=True,
                    tile_position=(0, 32 * grp),
                )

            for w in range(W):
                lo, hi = max(w - 2, 0), min(w, OW - 1)
                if lo > hi:
                    continue
                mid = (lo // PSUM_POS) * PSUM_POS + PSUM_POS - 1
                if hi <= mid:
                    emit(w, lo, hi)
                else:  # straddles a PSUM bank/group line
                    emit(w, lo, mid)
                    emit(w, mid + 1, hi)

            for grp in range(ngrp):
                npos = min(PSUM_POS, OW - grp * PSUM_POS)
                dst = rowbuf[:, grp * PSUM_POS * F
                             : grp * PSUM_POS * F + npos * F]
                src = pst[32 * grp:32 * grp + B, : npos * F]
                if grp % 2 == 0:
                    nc.vector.tensor_scalar_max(dst, src, 0.0)
                else:
                    nc.scalar.activation(
                        dst, src, mybir.ActivationFunctionType.Relu,
                    )
            rowbufs.append(rowbuf)

        # y stores ride the tails of the two HWDGE queues (emitted after
        # all weight DMAs in program order, so they can never head-of-line
        # block a weight prefetch), balancing queue byte loads:
        # sync/scalar 2.79MB each, gpsimd (x only) 3.07MB
        for oh in range(RPC):
            yeng = nc.sync if oh % 2 == 0 else nc.scalar
            yeng.dma_start(out=yout[oh], in_=rowbufs[oh])

    nc.compile()
    return nc


def _get_program():
    if "nc" not in _CACHE:
        _CACHE["nc"] = _build_program()
    return _CACHE["nc"]


def _prep_inputs(x, gamma, beta, moving_mean, moving_var, weight):
    """Host-side shard/layout/cast/quantize prep. Returns per-core in_maps."""
    import ml_dtypes

    x = np.asarray(x, dtype=np.float32)
    weight = np.asarray(weight, dtype=np.float32)

    # x: [B,H,W,C] -> pad H to 66 -> transpose to (h, c, w, b), fp16
    xpad = np.zeros((B, H + 2, W, C), np.float32)
    xpad[:, :H] = x
    xt_all = np.ascontiguousarray(xpad.transpose(1, 3, 2, 0)).astype(np.float16)

    # global symmetric E3M4 scale for the weight
    sw = np.float32(np.abs(weight).max() / FP8_MAX)
    if sw == 0.0:
        sw = np.float32(1.0)
    wq = (weight / sw).astype(ml_dtypes.float8_e3m4)
    wq = wq.reshape(OH, OW, KH, KW, C, F)

    # -> (oh, i, c, ow, j, f) then packed slots (w, g): ow=w-2+g, tap j=2-g
    wtr = np.ascontiguousarray(wq.transpose(0, 2, 4, 1, 3, 5))
    wp = np.zeros((OHP, KH, C, NSLOT, F), ml_dtypes.float8_e3m4)
    for w in range(W):
        for g in range(_GMIN[w], _GMAX[w] + 1):
            j = 2 - g
            ow = w - 2 + g
            slot = _SLOT_BASE[w] + (g - _GMIN[w])
            wp[:OH, :, :, slot, :] = wtr[:, :, :, ow, j, :]

    # rotate the K blocks per row: tap i of local row oh sits at partition
    # block (oh + i) mod 3, matching the rotating x stores
    wg = np.zeros_like(wp)
    oh_l = np.arange(OHP) % RPC
    rows = np.arange(OHP)
    for i in range(KH):
        wg[rows, (oh_l + i) % 3] = wp[rows, i]

    p96 = np.tile(
        np.stack([gamma, beta, moving_mean, moving_var,
                  np.full_like(gamma, sw)], axis=1).astype(np.float32),
        (KH, 1),
    )  # [96, 5]

    in_maps = []
    for k in range(8):
        R = k * RPC
        xc = np.ascontiguousarray(xt_all[R: R + RPC + 2].reshape(
            RPC + 2, C, XFREE))  # unique local rows 0..9
        wc = np.ascontiguousarray(wg[R: R + RPC]).reshape(RPC, NPART, WSLOT)
        in_maps.append({"xin": xc, "win": wc, "pin": p96})
    return in_maps


def _assemble_output(results):
    """results: list (per core) of {"yout": [RPC, B, YFREE] f16} -> [B,OH,OW,F] f32."""
    yall = np.concatenate([r["yout"] for r in results], axis=0)  # [nrows, B, YFREE]
    y = yall.astype(np.float32).reshape(-1, B, OW, F).transpose(1, 0, 2, 3)
    return np.ascontiguousarray(y[:, :OH] if y.shape[1] >= OH else y)


def run(inputs, trace=False, trace_cores=None):
    """Build/compile/run on 8 cores. Returns (y, BassKernelResults)."""
    from concourse.bass_utils import run_bass_kernel_spmd

    nc = _get_program()
    in_maps = _prep_inputs(**inputs)
    res = run_bass_kernel_spmd(
        nc,
        in_maps,
        core_ids=list(range(8)),
        trace=trace,
        **({"trace_cores": trace_cores} if trace_cores is not None else {}),
    )
    return _assemble_output(res.results), res


def kernel(x, gamma, beta, moving_mean, moving_var, weight):
    y, _ = run(
        dict(x=x, gamma=gamma, beta=beta, moving_mean=moving_mean,
             moving_var=moving_var, weight=weight)
    )
    return y


# revision 30
# speedup vs baseline: 1.0197x; 1.0197x over previous
"""Trainium2 Bass kernel for nn_BlockRF (BatchNorm -> LocallyConnected2D 3x3 valid -> ReLU).

Shapes (hardcoded per the problem spec):
  x:      [B=32, H=64, W=64, C=32]  f32
  gamma/beta/moving_mean/moving_var: [C=32] f32
  weight: [OH*OW=3844, KH*KW*C=288, F=32] f32
  out:    [B=32, OH=62, OW=62, F=32] f32

Strategy: shard over output rows (OH). OH=62 padded to 64 -> 8 rows/core on 8
cores. Each core streams only its slice of the (dominant) weight tensor.

The binding resource is SDMA byte throughput (~200 GB/s/core on SBUF-side
bytes, measured), so the weight stream is stored fp8 E3M4 (float8e3) with one
global symmetric scale sw (host quantized) and consumed directly by a
mixed-precision matmul (fp16 stationary x, fp8e3 moving weights — HW
supported, bit-exact). sw is folded into the BN affine params
(xn_scaled = sw*(A*x+Bb)), so the matmul directly produces y and no dequant
op exists anywhere.

Per core, per output row oh (pipelined via tile pools):
  - x rows r0..r0+2 live in an SBUF tile X[(i,c)=96, (w,b)=2048] fp16, BN
    (pre-scaled by sw) applied by VectorE; x loads and output stores ride the
    gpsimd SWDGE ring (no gpsimd tensor ops - they collide with DVE SBUF
    access and slow both engines).
  - weights stream per-oh: W[(i,c)=96, (w,g,f)=5952] fp8e3, slot (w,g) holds
    the 3x32-channel chunk j=2-g of position ow=w-2+g; each load is split
    across the sync and scalar HWDGE rings.
  - For each position ow: 3 accumulating matmuls (K=96, M=B=32, N<=3*F);
    lhsT = X w-slice (stationary fp16), rhs = weight chunk (moving fp8e3),
    fp32 PSUM accumulation.
  - PSUM: ONE [128, 512] single-bank tile per oh; the 4 position groups of
    16 live at partition offsets 0/32/64/96 (PE col tiling, tile_position
    auto-derived from out.base_partition). One [128,512] memset replaces two
    expensive 32-partition memsets, and matmuls of different groups can
    overlap in the PE array.
  - ReLU evacuation (fp16 out): 4 slices [32,512], alternating Vector/Scalar.
  - a warmup burst of K=1 zero-matmuls at kernel start lifts the PE HAM
    clock gate (1.2 -> 2.4 GHz) while the first DMAs stream.

Host side only pads/transposes/casts/quantizes (layout prep + sharding) - all
model arithmetic (BN, conv, ReLU) runs on device.
"""

import numpy as np

B, H, W, C, F = 32, 64, 64, 32, 32
KH = KW = 3
OH = OW = 62
OHP = 64  # padded OH
RPC = OHP // 8  # output rows per core = 8
EPS = 1e-3
NPART = KH * C  # 96 partitions = (i, c)
XFREE = W * B  # 2048
FP8_MAX = 15.5  # max finite |value| of E3M4
WARMUP_MM = 4  # K=96 N=512 zero-matmuls to lift the HAM clock gate
# packed weight slots: for stationary w-slice, valid g values are those with
# ow = w-2+g in [0, OW); slots stored g-ascending, w-major
_GMIN = [max(0, 2 - w) for w in range(W)]
_GMAX = [min(2, OW - 1 - (w - 2)) for w in range(W)]
_SLOT_BASE = [0] * W
for _w in range(1, W):
    _SLOT_BASE[_w] = _SLOT_BASE[_w - 1] + (_GMAX[_w - 1] - _GMIN[_w - 1] + 1)
NSLOT = _SLOT_BASE[-1] + (_GMAX[-1] - _GMIN[-1] + 1)  # 186
WSLOT = NSLOT * F  # 5952
YFREE = OW * F  # 1984
PSUM_POS = 16  # positions per PSUM partition-group (16*32*4B = 2KB = 1 bank)

_CACHE = {}


def _build_program():
    import concourse.mybir as mybir
    import concourse.tile as tile
    from concourse import bacc
    from contextlib import ExitStack

    f16 = mybir.dt.float16
    f32 = mybir.dt.float32
    f8 = mybir.dt.float8e3

    nc = bacc.Bacc("TRN2", target_bir_lowering=False, debug=False, num_devices=8)

    xin = nc.dram_tensor("xin", [RPC, NPART, XFREE], f16, kind="ExternalInput").ap()
    win = nc.dram_tensor("win", [RPC, NPART, WSLOT], f8, kind="ExternalInput").ap()
    pin = nc.dram_tensor("pin", [NPART, 5], f32, kind="ExternalInput").ap()
    yout = nc.dram_tensor("yout", [RPC, B, YFREE], f16, kind="ExternalOutput").ap()

    ngrp = (OW + PSUM_POS - 1) // PSUM_POS  # 4 groups of <=16 positions

    with ExitStack() as ctx:
        tc = ctx.enter_context(tile.TileContext(nc))
        singles = ctx.enter_context(tc.tile_pool(name="singles", bufs=1))
        xpool = ctx.enter_context(tc.tile_pool(name="xpool", bufs=8))
        xnpool = ctx.enter_context(tc.tile_pool(name="xnpool", bufs=3))
        wpool = ctx.enter_context(tc.tile_pool(name="wpool", bufs=12))
        opool = ctx.enter_context(tc.tile_pool(name="opool", bufs=8))
        pspool = ctx.enter_context(
            tc.tile_pool(name="pspool", bufs=6, space="PSUM")
        )

        # ---- BN affine params scaled by the global weight dequant scale sw
        # (pin col 4): A = sw*gamma/sqrt(var+eps), Bb = sw*(beta - mean*A0)
        par = singles.tile([NPART, 5], f32)
        nc.sync.dma_start(out=par, in_=pin)

        # x loads ride the gpsimd (SWDGE) queue alone (no y stores behind
        # them that could head-of-line-block the FIFO); all 8 issued up
        # front so the queue never starves
        xts = []
        for oh in range(RPC):
            xt = xpool.tile([NPART, XFREE], f16, name="xt", tag="xt")
            nc.gpsimd.dma_start(out=xt, in_=xin[oh])
            xts.append(xt)

        # PE warmup: K=96 zero-matmuls keep the PE array visibly busy while
        # the first DMAs stream, so the HAM clock gate opens (1.2 -> 2.4
        # GHz) before the real matmuls start (K=1 matmuls do NOT register
        # enough PE activity to trip the monitor)
        zt = singles.tile([NPART, 512], f16)
        nc.vector.memset(zt, 0.0)
        wups = pspool.tile([128, 512], mybir.dt.float32, name="wup", tag="ps")
        for _ in range(WARMUP_MM):
            nc.tensor.matmul(
                wups[:B], zt[:, :B], zt, start=True, stop=True,
                skip_group_check=True,
            )

        tmp = singles.tile([NPART, 1], f32)
        A = singles.tile([NPART, 1], f32)
        Bb = singles.tile([NPART, 1], f32)
        nc.vector.tensor_scalar_add(tmp, par[:, 3:4], EPS)  # var + eps
        nc.scalar.sqrt(tmp, tmp)
        nc.vector.reciprocal(A, tmp)  # 1/sqrt(var+eps)
        nc.vector.tensor_mul(A, A, par[:, 0:1])  # * gamma
        nc.vector.tensor_mul(tmp, A, par[:, 2:3])  # mean * A
        nc.vector.tensor_sub(Bb, par[:, 1:2], tmp)  # beta - mean*A
        nc.vector.tensor_mul(A, A, par[:, 4:5])  # * sw
        nc.vector.tensor_mul(Bb, Bb, par[:, 4:5])  # * sw

        HW = WSLOT // 2  # = 93*F: exactly the slots of w<32
        rowbufs = []
        for oh in range(RPC):
            # two weight tiles per row (w<32 / w>=32), loaded SEQUENTIALLY
            # on one HWDGE ring (even rows on sync, odd on scalar): the
            # front tile lands half a period earlier, so each row's matmul
            # burst splits into two sub-bursts ~2.2us apart - the PE HAM
            # activity window never sees a long idle stretch (which would
            # re-throttle the clock to 1.2 GHz), and the first matmuls of
            # each row start earlier
            weng = nc.sync if oh % 2 == 0 else nc.scalar
            wta = wpool.tile([NPART, HW], f8, name="wta", tag="wt")
            wtb = wpool.tile([NPART, WSLOT - HW], f8, name="wtb", tag="wt")
            weng.dma_start(out=wta, in_=win[oh][:, :HW])
            weng.dma_start(out=wtb, in_=win[oh][:, HW:])
            # ONE single-bank PSUM tile; position group g lives at
            # partition offset 32g (PE col tiling). PSUM 'start=True'
            # pend-zeroes a whole 2KB bank, so interleaved accumulation
            # slices cannot use it: memset the tile instead and accumulate
            # every matmul (start=False onto zeroed values). The memset is
            # emitted BEFORE the BN so it never serializes the
            # BN -> first-matmul critical path on the vector engine.
            pst = pspool.tile([128, PSUM_POS * F], mybir.dt.float32,
                              name="ps", tag="ps")
            nc.vector.memset(pst, 0.0)

            xt = xts[oh]
            xn = xnpool.tile([NPART, XFREE], f16)
            nc.vector.tensor_scalar(
                xn, xt, A, Bb,
                op0=mybir.AluOpType.mult, op1=mybir.AluOpType.add,
            )

            rowbuf = opool.tile([B, YFREE], f16)

            def emit(w, ow_lo, ow_hi):
                # one matmul covering positions ow_lo..ow_hi (inclusive) at
                # stationary w-slice; slots (w, g=2-(w-ow)) are
                # free-contiguous for ascending ow
                grp = ow_lo // PSUM_POS
                s = ow_lo - grp * PSUM_POS
                n = ow_hi - ow_lo + 1
                g_lo = 2 - (w - ow_lo)
                slot = _SLOT_BASE[w] + (g_lo - _GMIN[w])
                if slot * F < HW:
                    wsrc = wta[:, slot * F:(slot + n) * F]
                else:
                    wsrc = wtb[:, slot * F - HW:(slot + n) * F - HW]
                nc.tensor.matmul(
                    pst[32 * grp:32 * grp + B, s * F:(s + n) * F],
                    xn[:, w * B:(w + 1) * B],
                    wsrc,
                    start=False,
                    stop=True,
                    skip_group_check=True,
                    tile_position=(0, 32 * grp),
                )

            for w in range(W):
                lo, hi = max(w - 2, 0), min(w, OW - 1)
                if lo > hi:
                    continue
                mid = (lo // PSUM_POS) * PSUM_POS + PSUM_POS - 1
                if hi <= mid:
                    emit(w, lo, hi)
                else:  # straddles a PSUM bank/group line
                    emit(w, lo, mid)
                    emit(w, mid + 1, hi)

            for grp in range(ngrp):
                npos = min(PSUM_POS, OW - grp * PSUM_POS)
                dst = rowbuf[:, grp * PSUM_POS * F
                             : grp * PSUM_POS * F + npos * F]
                src = pst[32 * grp:32 * grp + B, : npos * F]
                if grp % 2 == 0:
                    nc.vector.tensor_scalar_max(dst, src, 0.0)
                else:
                    nc.scalar.activation(
                        dst, src, mybir.ActivationFunctionType.Relu,
                    )
            rowbufs.append(rowbuf)

        # y stores ride the tails of the two HWDGE queues (emitted after
        # all weight DMAs in program order, so they can never head-of-line
        # block a weight prefetch), balancing queue byte loads:
        # sync/scalar 2.79MB each, gpsimd (x only) 3.07MB
        for oh in range(RPC):
            yeng = nc.sync if oh % 2 == 0 else nc.scalar
            yeng.dma_start(out=yout[oh], in_=rowbufs[oh])

    nc.compile()
    return nc


def _get_program():
    if "nc" not in _CACHE:
        _CACHE["nc"] = _build_program()
    return _CACHE["nc"]


def _prep_inputs(x, gamma, beta, moving_mean, moving_var, weight):
    """Host-side shard/layout/cast/quantize prep. Returns per-core in_maps."""
    import ml_dtypes

    x = np.asarray(x, dtype=np.float32)
    weight = np.asarray(weight, dtype=np.float32)

    # x: [B,H,W,C] -> pad H to 66 -> transpose to (h, c, w, b), fp16
    xpad = np.zeros((B, H + 2, W, C), np.float32)
    xpad[:, :H] = x
    xt_all = np.ascontiguousarray(xpad.transpose(1, 3, 2, 0)).astype(np.float16)

    # global symmetric E3M4 scale for the weight
    sw = np.float32(np.abs(weight).max() / FP8_MAX)
    if sw == 0.0:
        sw = np.float32(1.0)
    wq = (weight / sw).astype(ml_dtypes.float8_e3m4)
    wq = wq.reshape(OH, OW, KH, KW, C, F)

    # -> (oh, i, c, ow, j, f) then packed slots (w, g): ow=w-2+g, tap j=2-g
    wtr = np.ascontiguousarray(wq.transpose(0, 2, 4, 1, 3, 5))
    wg = np.zeros((OHP, KH, C, NSLOT, F), ml_dtypes.float8_e3m4)
    for w in range(W):
        for g in range(_GMIN[w], _GMAX[w] + 1):
            j = 2 - g
            ow = w - 2 + g
            slot = _SLOT_BASE[w] + (g - _GMIN[w])
            wg[:OH, :, :, slot, :] = wtr[:, :, :, ow, j, :]

    p96 = np.tile(
        np.stack([gamma, beta, moving_mean, moving_var,
                  np.full_like(gamma, sw)], axis=1).astype(np.float32),
        (KH, 1),
    )  # [96, 5]

    in_maps = []
    for k in range(8):
        R = k * RPC
        xc = np.stack(
            [xt_all[R + oh: R + oh + 3].reshape(NPART, XFREE) for oh in range(RPC)]
        )  # [8, 96, 2048]
        wc = np.ascontiguousarray(wg[R: R + RPC]).reshape(RPC, NPART, WSLOT)
        in_maps.append({"xin": xc, "win": wc, "pin": p96})
    return in_maps


def _assemble_output(results):
    """results: list (per core) of {"yout": [RPC, B, YFREE] f16} -> [B,OH,OW,F] f32."""
    yall = np.concatenate([r["yout"] for r in results], axis=0)  # [nrows, B, YFREE]
    y = yall.astype(np.float32).reshape(-1, B, OW, F).transpose(1, 0, 2, 3)
    return np.ascontiguousarray(y[:, :OH] if y.shape[1] >= OH else y)


def run(inputs, trace=False, trace_cores=None):
    """Build/compile/run on 8 cores. Returns (y, BassKernelResults)."""
    from concourse.bass_utils import run_bass_kernel_spmd

    nc = _get_program()
    in_maps = _prep_inputs(**inputs)
    res = run_bass_kernel_spmd(
        nc,
        in_maps,
        core_ids=list(range(8)),
        trace=trace,
        **({"trace_cores": trace_cores} if trace_cores is not None else {}),
    )
    return _assemble_output(res.results), res


def kernel(x, gamma, beta, moving_mean, moving_var, weight):
    y, _ = run(
        dict(x=x, gamma=gamma, beta=beta, moving_mean=moving_mean,
             moving_var=moving_var, weight=weight)
    )
    return y


# revision 31
# speedup vs baseline: 1.0266x; 1.0068x over previous
"""Trainium2 Bass kernel for nn_BlockRF (BatchNorm -> LocallyConnected2D 3x3 valid -> ReLU).

Shapes (hardcoded per the problem spec):
  x:      [B=32, H=64, W=64, C=32]  f32
  gamma/beta/moving_mean/moving_var: [C=32] f32
  weight: [OH*OW=3844, KH*KW*C=288, F=32] f32
  out:    [B=32, OH=62, OW=62, F=32] f32

Strategy: shard over output rows (OH). OH=62 padded to 64 -> 8 rows/core on 8
cores. Each core streams only its slice of the (dominant) weight tensor.

The binding resource is SDMA byte throughput (~200 GB/s/core on SBUF-side
bytes, measured), so the weight stream is stored fp8 E3M4 (float8e3) with one
global symmetric scale sw (host quantized) and consumed directly by a
mixed-precision matmul (fp16 stationary x, fp8e3 moving weights — HW
supported, bit-exact). sw is folded into the BN affine params
(xn_scaled = sw*(A*x+Bb)), so the matmul directly produces y and no dequant
op exists anywhere.

Per core, per output row oh (pipelined via tile pools):
  - x rows r0..r0+2 live in an SBUF tile X[(i,c)=96, (w,b)=2048] fp16, BN
    (pre-scaled by sw) applied by VectorE; x loads and output stores ride the
    gpsimd SWDGE ring (no gpsimd tensor ops - they collide with DVE SBUF
    access and slow both engines).
  - weights stream per-oh: W[(i,c)=96, (w,g,f)=5952] fp8e3, slot (w,g) holds
    the 3x32-channel chunk j=2-g of position ow=w-2+g; each load is split
    across the sync and scalar HWDGE rings.
  - For each position ow: 3 accumulating matmuls (K=96, M=B=32, N<=3*F);
    lhsT = X w-slice (stationary fp16), rhs = weight chunk (moving fp8e3),
    fp32 PSUM accumulation.
  - PSUM: ONE [128, 512] single-bank tile per oh; the 4 position groups of
    16 live at partition offsets 0/32/64/96 (PE col tiling, tile_position
    auto-derived from out.base_partition). One [128,512] memset replaces two
    expensive 32-partition memsets, and matmuls of different groups can
    overlap in the PE array.
  - ReLU evacuation (fp16 out): 4 slices [32,512], alternating Vector/Scalar.
  - a warmup burst of K=1 zero-matmuls at kernel start lifts the PE HAM
    clock gate (1.2 -> 2.4 GHz) while the first DMAs stream.

Host side only pads/transposes/casts/quantizes (layout prep + sharding) - all
model arithmetic (BN, conv, ReLU) runs on device.
"""

import numpy as np

B, H, W, C, F = 32, 64, 64, 32, 32
KH = KW = 3
OH = OW = 62
OHP = 64  # padded OH
RPC = OHP // 8  # output rows per core = 8
EPS = 1e-3
NPART = KH * C  # 96 partitions = (i, c)
XFREE = W * B  # 2048
FP8_MAX = 15.5  # max finite |value| of E3M4
WARMUP_MM = 4  # K=96 N=512 zero-matmuls to lift the HAM clock gate
# packed weight slots: for stationary w-slice, valid g values are those with
# ow = w-2+g in [0, OW); slots stored g-ascending, w-major
_GMIN = [max(0, 2 - w) for w in range(W)]
_GMAX = [min(2, OW - 1 - (w - 2)) for w in range(W)]
_SLOT_BASE = [0] * W
for _w in range(1, W):
    _SLOT_BASE[_w] = _SLOT_BASE[_w - 1] + (_GMAX[_w - 1] - _GMIN[_w - 1] + 1)
NSLOT = _SLOT_BASE[-1] + (_GMAX[-1] - _GMIN[-1] + 1)  # 186
WSLOT = NSLOT * F  # 5952
YFREE = OW * F  # 1984
PSUM_POS = 16  # positions per PSUM partition-group (16*32*4B = 2KB = 1 bank)

_CACHE = {}


def _build_program():
    import concourse.mybir as mybir
    import concourse.tile as tile
    from concourse import bacc
    from contextlib import ExitStack

    f16 = mybir.dt.float16
    f32 = mybir.dt.float32
    f8 = mybir.dt.float8e3

    nc = bacc.Bacc("TRN2", target_bir_lowering=False, debug=False, num_devices=8)

    xin = nc.dram_tensor("xin", [RPC, NPART, XFREE], f16, kind="ExternalInput").ap()
    win = nc.dram_tensor("win", [RPC, NPART, WSLOT], f8, kind="ExternalInput").ap()
    pin = nc.dram_tensor("pin", [NPART, 5], f32, kind="ExternalInput").ap()
    yout = nc.dram_tensor("yout", [RPC, B, YFREE], f16, kind="ExternalOutput").ap()

    ngrp = (OW + PSUM_POS - 1) // PSUM_POS  # 4 groups of <=16 positions

    with ExitStack() as ctx:
        tc = ctx.enter_context(tile.TileContext(nc))
        singles = ctx.enter_context(tc.tile_pool(name="singles", bufs=1))
        xpool = ctx.enter_context(tc.tile_pool(name="xpool", bufs=8))
        xnpool = ctx.enter_context(tc.tile_pool(name="xnpool", bufs=3))
        wpool = ctx.enter_context(tc.tile_pool(name="wpool", bufs=12))
        opool = ctx.enter_context(tc.tile_pool(name="opool", bufs=8))
        pspool = ctx.enter_context(
            tc.tile_pool(name="pspool", bufs=6, space="PSUM")
        )

        # ---- BN affine params scaled by the global weight dequant scale sw
        # (pin col 4): A = sw*gamma/sqrt(var+eps), Bb = sw*(beta - mean*A0)
        par = singles.tile([NPART, 5], f32)
        nc.sync.dma_start(out=par, in_=pin)

        # x loads ride the gpsimd (SWDGE) queue alone (no y stores behind
        # them that could head-of-line-block the FIFO); all 8 issued up
        # front so the queue never starves
        xts = []
        for oh in range(RPC):
            xt = xpool.tile([NPART, XFREE], f16, name="xt", tag="xt")
            nc.gpsimd.dma_start(out=xt, in_=xin[oh])
            xts.append(xt)

        # PE warmup: K=96 zero-matmuls keep the PE array visibly busy while
        # the first DMAs stream, so the HAM clock gate opens (1.2 -> 2.4
        # GHz) before the real matmuls start (K=1 matmuls do NOT register
        # enough PE activity to trip the monitor)
        zt = singles.tile([NPART, 512], f16)
        nc.vector.memset(zt, 0.0)
        wups = pspool.tile([128, 512], mybir.dt.float32, name="wup", tag="ps")
        for _ in range(WARMUP_MM):
            nc.tensor.matmul(
                wups[:B], zt[:, :B], zt, start=True, stop=True,
                skip_group_check=True,
            )

        tmp = singles.tile([NPART, 1], f32)
        A = singles.tile([NPART, 1], f32)
        Bb = singles.tile([NPART, 1], f32)
        nc.vector.tensor_scalar_add(tmp, par[:, 3:4], EPS)  # var + eps
        nc.scalar.sqrt(tmp, tmp)
        nc.vector.reciprocal(A, tmp)  # 1/sqrt(var+eps)
        nc.vector.tensor_mul(A, A, par[:, 0:1])  # * gamma
        nc.vector.tensor_mul(tmp, A, par[:, 2:3])  # mean * A
        nc.vector.tensor_sub(Bb, par[:, 1:2], tmp)  # beta - mean*A
        nc.vector.tensor_mul(A, A, par[:, 4:5])  # * sw
        nc.vector.tensor_mul(Bb, Bb, par[:, 4:5])  # * sw

        HW = WSLOT // 2  # = 93*F: exactly the slots of w<32
        rowbufs = []
        for oh in range(RPC):
            # two weight tiles per row (w<32 / w>=32), loaded SEQUENTIALLY
            # on one HWDGE ring (even rows on sync, odd on scalar): the
            # front tile lands half a period earlier, so each row's matmul
            # burst splits into two sub-bursts ~2.2us apart - the PE HAM
            # activity window never sees a long idle stretch (which would
            # re-throttle the clock to 1.2 GHz), and the first matmuls of
            # each row start earlier
            wta = wpool.tile([NPART, HW], f8, name="wta", tag="wt")
            wtb = wpool.tile([NPART, WSLOT - HW], f8, name="wtb", tag="wt")
            if oh < RPC - 2:
                weng = nc.sync if oh % 2 == 0 else nc.scalar
                weng.dma_start(out=wta, in_=win[oh][:, :HW])
                weng.dma_start(out=wtb, in_=win[oh][:, HW:])
            else:
                # last two rows: halves in parallel across both rings
                # (crossed for byte balance) so the final inputs land
                # earlier and the tail shortens
                ea = nc.sync if oh % 2 == 0 else nc.scalar
                eb = nc.scalar if oh % 2 == 0 else nc.sync
                ea.dma_start(out=wta, in_=win[oh][:, :HW])
                eb.dma_start(out=wtb, in_=win[oh][:, HW:])
            # ONE single-bank PSUM tile; position group g lives at
            # partition offset 32g (PE col tiling). PSUM 'start=True'
            # pend-zeroes a whole 2KB bank, so interleaved accumulation
            # slices cannot use it: memset the tile instead and accumulate
            # every matmul (start=False onto zeroed values). The memset is
            # emitted BEFORE the BN so it never serializes the
            # BN -> first-matmul critical path on the vector engine.
            pst = pspool.tile([128, PSUM_POS * F], mybir.dt.float32,
                              name="ps", tag="ps")
            nc.vector.memset(pst, 0.0)

            xt = xts[oh]
            xn = xnpool.tile([NPART, XFREE], f16)
            nc.vector.tensor_scalar(
                xn, xt, A, Bb,
                op0=mybir.AluOpType.mult, op1=mybir.AluOpType.add,
            )

            rowbuf = opool.tile([B, YFREE], f16)

            def emit(w, ow_lo, ow_hi):
                # one matmul covering positions ow_lo..ow_hi (inclusive) at
                # stationary w-slice; slots (w, g=2-(w-ow)) are
                # free-contiguous for ascending ow
                grp = ow_lo // PSUM_POS
                s = ow_lo - grp * PSUM_POS
                n = ow_hi - ow_lo + 1
                g_lo = 2 - (w - ow_lo)
                slot = _SLOT_BASE[w] + (g_lo - _GMIN[w])
                if slot * F < HW:
                    wsrc = wta[:, slot * F:(slot + n) * F]
                else:
                    wsrc = wtb[:, slot * F - HW:(slot + n) * F - HW]
                nc.tensor.matmul(
                    pst[32 * grp:32 * grp + B, s * F:(s + n) * F],
                    xn[:, w * B:(w + 1) * B],
                    wsrc,
                    start=False,
                    stop=True,
                    skip_group_check=True,
                    tile_position=(0, 32 * grp),
                )

            for w in range(W):
                lo, hi = max(w - 2, 0), min(w, OW - 1)
                if lo > hi:
                    continue
                mid = (lo // PSUM_POS) * PSUM_POS + PSUM_POS - 1
                if hi <= mid:
                    emit(w, lo, hi)
                else:  # straddles a PSUM bank/group line
                    emit(w, lo, mid)
                    emit(w, mid + 1, hi)

            for grp in range(ngrp):
                npos = min(PSUM_POS, OW - grp * PSUM_POS)
                dst = rowbuf[:, grp * PSUM_POS * F
                             : grp * PSUM_POS * F + npos * F]
                src = pst[32 * grp:32 * grp + B, : npos * F]
                if grp % 2 == 0:
                    nc.vector.tensor_scalar_max(dst, src, 0.0)
                else:
                    nc.scalar.activation(
                        dst, src, mybir.ActivationFunctionType.Relu,
                    )
            rowbufs.append(rowbuf)

        # y stores ride the tails of the two HWDGE queues (emitted after
        # all weight DMAs in program order, so they can never head-of-line
        # block a weight prefetch), balancing queue byte loads:
        # sync/scalar 2.79MB each, gpsimd (x only) 3.07MB
        for oh in range(RPC):
            yeng = nc.sync if oh % 2 == 0 else nc.scalar
            yeng.dma_start(out=yout[oh], in_=rowbufs[oh])

    nc.compile()
    return nc


def _get_program():
    if "nc" not in _CACHE:
        _CACHE["nc"] = _build_program()
    return _CACHE["nc"]


def _prep_inputs(x, gamma, beta, moving_mean, moving_var, weight):
    """Host-side shard/layout/cast/quantize prep. Returns per-core in_maps."""
    import ml_dtypes

    x = np.asarray(x, dtype=np.float32)
    weight = np.asarray(weight, dtype=np.float32)

    # x: [B,H,W,C] -> pad H to 66 -> transpose to (h, c, w, b), fp16
    xpad = np.zeros((B, H + 2, W, C), np.float32)
    xpad[:, :H] = x
    xt_all = np.ascontiguousarray(xpad.transpose(1, 3, 2, 0)).astype(np.float16)

    # global symmetric E3M4 scale for the weight
    sw = np.float32(np.abs(weight).max() / FP8_MAX)
    if sw == 0.0:
        sw = np.float32(1.0)
    wq = (weight / sw).astype(ml_dtypes.float8_e3m4)
    wq = wq.reshape(OH, OW, KH, KW, C, F)

    # -> (oh, i, c, ow, j, f) then packed slots (w, g): ow=w-2+g, tap j=2-g
    wtr = np.ascontiguousarray(wq.transpose(0, 2, 4, 1, 3, 5))
    wg = np.zeros((OHP, KH, C, NSLOT, F), ml_dtypes.float8_e3m4)
    for w in range(W):
        for g in range(_GMIN[w], _GMAX[w] + 1):
            j = 2 - g
            ow = w - 2 + g
            slot = _SLOT_BASE[w] + (g - _GMIN[w])
            wg[:OH, :, :, slot, :] = wtr[:, :, :, ow, j, :]

    p96 = np.tile(
        np.stack([gamma, beta, moving_mean, moving_var,
                  np.full_like(gamma, sw)], axis=1).astype(np.float32),
        (KH, 1),
    )  # [96, 5]

    in_maps = []
    for k in range(8):
        R = k * RPC
        xc = np.stack(
            [xt_all[R + oh: R + oh + 3].reshape(NPART, XFREE) for oh in range(RPC)]
        )  # [8, 96, 2048]
        wc = np.ascontiguousarray(wg[R: R + RPC]).reshape(RPC, NPART, WSLOT)
        in_maps.append({"xin": xc, "win": wc, "pin": p96})
    return in_maps


def _assemble_output(results):
    """results: list (per core) of {"yout": [RPC, B, YFREE] f16} -> [B,OH,OW,F] f32."""
    yall = np.concatenate([r["yout"] for r in results], axis=0)  # [nrows, B, YFREE]
    y = yall.astype(np.float32).reshape(-1, B, OW, F).transpose(1, 0, 2, 3)
    return np.ascontiguousarray(y[:, :OH] if y.shape[1] >= OH else y)


def run(inputs, trace=False, trace_cores=None):
    """Build/compile/run on 8 cores. Returns (y, BassKernelResults)."""
    from concourse.bass_utils import run_bass_kernel_spmd

    nc = _get_program()
    in_maps = _prep_inputs(**inputs)
    res = run_bass_kernel_spmd(
        nc,
        in_maps,
        core_ids=list(range(8)),
        trace=trace,
        **({"trace_cores": trace_cores} if trace_cores is not None else {}),
    )
    return _assemble_output(res.results), res


def kernel(x, gamma, beta, moving_mean, moving_var, weight):
    y, _ = run(
        dict(x=x, gamma=gamma, beta=beta, moving_mean=moving_mean,
             moving_var=moving_var, weight=weight)
    )
    return y
